# revision 32
# baseline (speedup 1.0000x reference)
# Trainium2 Bass kernel for nn_BinLinearEval:
#   out[b, o] = (round(x @ W.T + bias) * sign >= 0) ? 1.0 : 0.0
#
# Math folding (exact because bias is integer-valued and sign in {-1,+1}):
#   out = 1  iff  sign*(dot + bias) >= -0.5
#       = 1  iff  dot' >= thr_o      where dot' = x @ (sign.T*W).T  (W' still
#         ternary) and thr_o = -sign_o*bias_o - 0.5.
#
# Precision: x is shipped as an e4m3 hi + e4m3 residual*64 pair (2 B/elem)
# and BOTH passes run as fp8 DoubleRow matmuls. ~1700 threshold flips of
# 16.7M (rel err ~0.014 vs the 2e-2 gate).
#
# Measured facts this schedule is built on (NTFF traces):
#  - A DR FD=512 matmul stream paces at 216 ns/MM warm (2.4 GHz); when the
#    chip's P0 power state engages under sustained 8-core load it drops to
#    259 ns/MM (2.0 GHz) — chip-state dependent, not schedulable around.
#    LDWEIGHTS fully hides in the PE pull-ahead window at ANY weight-reuse
#    pattern, so no LDW amortization is needed.
#  - The framework preamble ends ~6.4 us; first DMA bytes move ~8-10 us.
#    Both HWDGE rings share the 16 SDMA engines round-robin per PACKET, so
#    each ring sustains ~185 GB/s while both are busy (~370 aggregate).
#    Descriptor size = per-partition contiguous run: transfers with tiny
#    per-partition runs (8B thr, 1KB chunks) waste whole ring turns.
#  - Receipts (sem>=16) land ~50 ns after transfer-done; what matters is
#    need-ordering of the two ring FIFOs and receipt granularity.
#  - A PE idle gap >3.4 us mid-stream re-throttles HAM (K=4/8, half clock)
#    for ~2 windows — a single late transfer can cascade into ~6 us lost.
# Schedule: ~4 us of warmup MMs on a memset tile (no DMA dependency) so
# HAM un-throttles before real data lands; the two oc-passes interleave
# per chunk so fresh-x demand (~296 GB/s) stays below ring supply; g0
# arrives as 4 quarter-DMAs, other groups as hi/lo halves split across
# both rings; thr is replicated x128 host-side for full descriptors; the
# last group runs its oc passes sequentially with its out split per-oc
# across the two then-idle rings to shorten the end-of-kernel receipt.

import os
from contextlib import ExitStack

import numpy as np
import ml_dtypes

BATCH, IN_F, OUT_F = 65536, 1024, 256
N_CORES = 8
B_CORE = BATCH // N_CORES  # 8192
P = 128
KC = IN_F // P             # 8 k-chunks of 128
NCH = KC                   # 8 DoubleRow chunk-steps: 4 hi + 4 lo, 256-contract each
OC = OUT_F // P            # 2 out-channel chunks
GRP = 512                  # batch tile (= max DR matmul moving dim / 2)
N_GROUPS = B_CORE // GRP   # 16
N_WARM = 10                # dummy MMs spanning ~4.3 us of PE-busy before data

_CACHE = {}


def _build():
    """Build (and cache) the Bass module. Returns the compiled nc."""
    if "nc" in _CACHE:
        return _CACHE["nc"]

    import concourse.bacc as bacc
    import concourse.mybir as mybir
    import concourse.tile as tile

    nc = bacc.Bacc(
        "TRN2",
        target_bir_lowering=False,
        debug=False,
        num_devices=N_CORES,
    )

    f32 = mybir.dt.float32
    f8 = mybir.dt.float8e4
    DR = mybir.MatmulPerfMode.DoubleRow

    # x8 chunk layout: [P, group, chunk(0:4 hi, 4:8 lo), j, GRP] where the
    # DoubleRow pair (chunk c, j) covers global k = (c%4)*256 + j*128 + p
    x8_d = nc.dram_tensor(
        "x8", [P, N_GROUPS, NCH, 2, GRP], f8, kind="ExternalInput"
    ).ap()
    # weights split by oc so each half is one contiguous 2KB/partition DMA
    w8_d = nc.dram_tensor("w8", [P, OC, NCH, 2, P], f8, kind="ExternalInput").ap()
    # thr replicated x128 on host: full 1KB/partition descriptors instead
    # of 8B ones (which waste ~3us of ring turns at packet round-robin)
    thr_d = nc.dram_tensor("thr", [P, OC, P], f32, kind="ExternalInput").ap()
    out_d = nc.dram_tensor(
        "out", [P, N_GROUPS, OC, GRP], f8, kind="ExternalOutput"
    ).ap()

    with tile.TileContext(nc) as tc, ExitStack() as ctx:
        const = ctx.enter_context(tc.tile_pool(name="const", bufs=1))
        io = ctx.enter_context(tc.tile_pool(name="io", bufs=1))
        outp = ctx.enter_context(tc.tile_pool(name="outp", bufs=1))
        psum = ctx.enter_context(tc.tile_pool(name="psum", bufs=8, space="PSUM"))

        w8_sb = const.tile([P, OC, NCH, 2, P], f8)
        thr_sb = const.tile([P, OC, P], f32)
        warm_x = const.tile([P, 2, GRP], f8)

        xt = {}
        for g in range(N_GROUPS):
            xt[g] = io.tile([P, NCH, 2, GRP], f8, name=f"x{g}", bufs=1)

        # warmup operand comes from one memset, not DMA, so the PE can
        # start burning its HAM ramp right after the preamble barrier
        nc.vector.memset(warm_x, 0.25)

        # ── DMA triggers, need-ordered across the two ring FIFOs ──
        # Both w8 halves go first, one per ring (the interleaved stream
        # needs oc0-c0 AND oc1-c0 immediately); g0 in quarters (2KB
        # descriptors, receipt per 2 chunks) so the stream can start while
        # g0 is still arriving; all other groups as hi/lo halves (4KB
        # descriptors) delivered in lockstep across the rings.
        H = NCH // 2
        engs = [nc.sync, nc.scalar]
        nc.sync.dma_start(out=w8_sb[:, 0], in_=w8_d[:, 0])
        nc.scalar.dma_start(out=w8_sb[:, 1], in_=w8_d[:, 1])
        for q in range(4):
            engs[q % 2].dma_start(
                out=xt[0][:, 2 * q : 2 * q + 2], in_=x8_d[:, 0, 2 * q : 2 * q + 2]
            )
        for g in range(1, N_GROUPS):
            nc.sync.dma_start(out=xt[g][:, :H], in_=x8_d[:, g, :H])
            nc.scalar.dma_start(out=xt[g][:, H:], in_=x8_d[:, g, H:])
            if g == 3:
                # thr needed only when g0's epilogue becomes psum-critical
                # (~25 us); late enough to stay off the critical supply path
                nc.scalar.dma_start(out=thr_sb, in_=thr_d)

        # ── PE warmup: data-independent DR MMs at cold pace (~0.43-0.52
        # us each) spanning ~4.3 us so HAM reaches K=8/8 before the first
        # real matmul. psum never read; slots recycle into the pool.
        wps = [psum.tile([P, GRP], f32, name="ps") for _ in range(2)]
        for i in range(N_WARM):
            nc.tensor.matmul(
                wps[i % 2], warm_x[:, :, :P], warm_x,
                start=True, stop=True, perf_mode=DR,
            )

        # ── main stream: 16 groups x 8 chunk-steps x 2 oc-passes ──
        # The oc passes are interleaved per chunk so fresh-x demand is a
        # steady ~296 GB/s (one 128KB chunk per 2 MMs) instead of 2x-supply
        # bursts during each oc0 pass — the rings then never fall behind.
        # The LAST group runs its oc passes sequentially so the oc0
        # epilogue + out drain ~1.7 us before the end (shorter tail), with
        # its out split per-oc across the two then-idle rings.
        for g in range(N_GROUPS):
            last = g == N_GROUPS - 1
            if not last:
                ob = outp.tile([P, OC, GRP], f8, name=f"ob{g}", bufs=1)
                pss = [psum.tile([P, GRP], f32, name="ps") for _ in range(OC)]
                for c in range(NCH):
                    for oc in range(OC):
                        nc.tensor.matmul(
                            pss[oc],
                            w8_sb[:, oc, c],
                            xt[g][:, c],
                            start=(c == 0),
                            stop=(c == NCH - 1),
                            perf_mode=DR,
                        )
                for oc in range(OC):
                    nc.vector.tensor_scalar(
                        ob[:, oc],
                        pss[oc],
                        thr_sb[:, oc, :1],
                        None,
                        mybir.AluOpType.is_ge,
                    )
                eng = nc.sync if g % 2 else nc.scalar
                eng.dma_start(out=out_d[:, g], in_=ob)
            else:
                ob_last = outp.tile([P, OC, GRP], f8, name="oblast", bufs=1)
                for oc in range(OC):
                    ps = psum.tile([P, GRP], f32, name="ps")
                    for c in range(NCH):
                        nc.tensor.matmul(
                            ps,
                            w8_sb[:, oc, c],
                            xt[g][:, c],
                            start=(c == 0),
                            stop=(c == NCH - 1),
                            perf_mode=DR,
                        )
                    if oc == 0:
                        nc.vector.tensor_scalar(
                            ob_last[:, 0],
                            ps,
                            thr_sb[:, 0, :1],
                            None,
                            mybir.AluOpType.is_ge,
                        )
                        nc.sync.dma_start(out=out_d[:, g, 0], in_=ob_last[:, 0])
                    else:
                        # final pass: halve the epilogue so the first out
                        # can trigger ~0.4us earlier and the two receipt
                        # round-trips overlap on the two idle rings
                        hg = GRP // 2
                        for h in range(2):
                            nc.vector.tensor_scalar(
                                ob_last[:, 1, h * hg : (h + 1) * hg],
                                ps[:, h * hg : (h + 1) * hg],
                                thr_sb[:, 1, :1],
                                None,
                                mybir.AluOpType.is_ge,
                            )
                            eng = nc.scalar if h == 0 else nc.sync
                            eng.dma_start(
                                out=out_d[:, g, 1, h * hg : (h + 1) * hg],
                                in_=ob_last[:, 1, h * hg : (h + 1) * hg],
                            )

    nc.compile()
    _CACHE["nc"] = nc
    return nc


def _prep_inputs(x, weight, bias, sign):
    """Host-side prep: fold sign into weights, build thresholds, split x into
    an e4m3 hi + e4m3 residual*64 pair in DoubleRow-interleaved layout."""
    f8np = ml_dtypes.float8_e4m3fn
    x = np.asarray(x, dtype=np.float32)
    weight = np.asarray(weight, dtype=np.float32)
    bias = np.asarray(bias, dtype=np.float32)
    sign = np.asarray(sign, dtype=np.float32).reshape(1, OUT_F)

    wp = sign.T * weight                      # [OUT_F, IN_F], ternary
    thr = (-sign[0] * bias - np.float32(0.5)).astype(np.float32)  # [OUT_F]
    thr2 = np.ascontiguousarray(
        np.repeat(thr.reshape(OC, P).T[:, :, None], P, axis=2)
    )  # [P, OC, P] replicated for full-size DMA descriptors

    # weights: [P, oc, chunk, j, 128]; chunks 0:4 = W' (ternary, exact in
    # e4m3), 4:8 = W'/64 (+-2^-6, exact in e4m3)
    wT = wp.T  # [IN_F, OUT_F]
    whi = wT.reshape(NCH // 2, 2, P, OUT_F).transpose(2, 0, 1, 3)
    wlo = (wT * np.float32(1.0 / 64.0)).reshape(NCH // 2, 2, P, OUT_F).transpose(
        2, 0, 1, 3
    )
    w8 = np.concatenate([whi, wlo], axis=1)   # [P, NCH, 2, OUT_F]
    w8 = np.ascontiguousarray(
        w8.reshape(P, NCH, 2, OC, P).transpose(0, 3, 1, 2, 4)
    ).astype(f8np)                            # [P, OC, NCH, 2, P]

    xhi8 = x.astype(f8np)
    xlo8 = ((x - xhi8.astype(np.float32)) * np.float32(64.0)).astype(f8np)

    in_maps = []
    for c in range(N_CORES):
        sl = slice(c * B_CORE, (c + 1) * B_CORE)
        hi = xhi8[sl].reshape(N_GROUPS, GRP, NCH // 2, 2, P).transpose(
            4, 0, 2, 3, 1
        )                                      # [P, g, 4, 2, GRP]
        lo = xlo8[sl].reshape(N_GROUPS, GRP, NCH // 2, 2, P).transpose(
            4, 0, 2, 3, 1
        )
        x8 = np.ascontiguousarray(np.concatenate([hi, lo], axis=2))
        in_maps.append({"x8": x8, "w8": w8, "thr": thr2})
    return in_maps


def _assemble(results):
    """[core][P, N_GROUPS, OC, GRP] fp8 -> [BATCH, OUT_F] fp32"""
    parts = []
    for r in results:
        a = (
            np.asarray(r["out"])
            .view(ml_dtypes.float8_e4m3fn)
            .astype(np.float32)
            .reshape(P, N_GROUPS, OC, GRP)
        )
        # out[b, o]: b = g*GRP + col, o = oc*P + p
        parts.append(
            a.transpose(1, 3, 2, 0).reshape(B_CORE, OUT_F)
        )
    return np.ascontiguousarray(np.concatenate(parts, axis=0))


def run(x, weight, bias, sign, trace=False):
    """Run the kernel; returns (output, BassKernelResults)."""
    from concourse.bass_utils import run_bass_kernel_spmd

    if not trace:
        os.environ["BASS_NEVER_TRACE"] = "1"
    else:
        os.environ.pop("BASS_NEVER_TRACE", None)

    nc = _build()
    in_maps = _prep_inputs(x, weight, bias, sign)
    res = run_bass_kernel_spmd(
        nc,
        in_maps,
        core_ids=list(range(N_CORES)),
        trace=trace,
    )
    return _assemble(res.results), res


def kernel(x, weight, bias, sign):
    out, _ = run(x, weight, bias, sign, trace=False)
    return out


# revision 33
# speedup vs baseline: 1.0377x; 1.0377x over previous
# Trainium2 Bass kernel for nn_BinLinearEval:
#   out[b, o] = (round(x @ W.T + bias) * sign >= 0) ? 1.0 : 0.0
#
# Math folding (exact because bias is integer-valued and sign in {-1,+1}):
#   out = 1  iff  sign*(dot + bias) >= -0.5
#       = 1  iff  dot' >= thr_o      where dot' = x @ (sign.T*W).T  (W' still
#         ternary) and thr_o = -sign_o*bias_o - 0.5.
#
# Precision: x is shipped as an e4m3 hi + e4m3 residual*64 pair (2 B/elem)
# and BOTH passes run as fp8 DoubleRow matmuls. ~1700 threshold flips of
# 16.7M (rel err ~0.014 vs the 2e-2 gate).
#
# Measured facts this schedule is built on (NTFF traces):
#  - A DR FD=512 matmul stream paces at 216 ns/MM warm (2.4 GHz); when the
#    chip's P0 power state engages under sustained 8-core load it drops to
#    259 ns/MM (2.0 GHz) — chip-state dependent, not schedulable around.
#    LDWEIGHTS fully hides in the PE pull-ahead window at ANY weight-reuse
#    pattern, so no LDW amortization is needed.
#  - The framework preamble ends ~6.4 us; first DMA bytes move ~8-10 us.
#    Both HWDGE rings share the 16 SDMA engines round-robin per PACKET, so
#    each ring sustains ~185 GB/s while both are busy (~370 aggregate).
#    Descriptor size = per-partition contiguous run: transfers with tiny
#    per-partition runs (8B thr, 1KB chunks) waste whole ring turns.
#  - Receipts (sem>=16) land ~50 ns after transfer-done; what matters is
#    need-ordering of the two ring FIFOs and receipt granularity.
#  - A PE idle gap >3.4 us mid-stream re-throttles HAM (K=4/8, half clock)
#    for ~2 windows — a single late transfer can cascade into ~6 us lost.
# Schedule: ~4 us of warmup MMs on a memset tile (no DMA dependency) so
# HAM un-throttles before real data lands; the two oc-passes interleave
# per chunk so fresh-x demand (~296 GB/s) stays below ring supply; g0
# arrives as 4 quarter-DMAs, other groups as hi/lo halves split across
# both rings; thr is replicated x128 host-side for full descriptors; the
# last group runs its oc passes sequentially with its out split per-oc
# across the two then-idle rings to shorten the end-of-kernel receipt.

import os
from contextlib import ExitStack

import numpy as np
import ml_dtypes

BATCH, IN_F, OUT_F = 65536, 1024, 256
N_CORES = 8
B_CORE = BATCH // N_CORES  # 8192
P = 128
KC = IN_F // P             # 8 k-chunks of 128
NCH = KC                   # 8 DoubleRow chunk-steps: 4 hi + 4 lo, 256-contract each
OC = OUT_F // P            # 2 out-channel chunks
GRP = 512                  # batch tile (= max DR matmul moving dim / 2)
N_GROUPS = B_CORE // GRP   # 16
N_WARM = 9                 # dummy MMs spanning ~4.3 us of PE-busy before data

_CACHE = {}


def _build():
    """Build (and cache) the Bass module. Returns the compiled nc."""
    if "nc" in _CACHE:
        return _CACHE["nc"]

    import concourse.bacc as bacc
    import concourse.mybir as mybir
    import concourse.tile as tile

    nc = bacc.Bacc(
        "TRN2",
        target_bir_lowering=False,
        debug=False,
        num_devices=N_CORES,
    )

    f32 = mybir.dt.float32
    f8 = mybir.dt.float8e4
    DR = mybir.MatmulPerfMode.DoubleRow

    # x8 chunk layout: [P, group, chunk(0:4 hi, 4:8 lo), j, GRP] where the
    # DoubleRow pair (chunk c, j) covers global k = (c%4)*256 + j*128 + p
    x8_d = nc.dram_tensor(
        "x8", [P, N_GROUPS, NCH, 2, GRP], f8, kind="ExternalInput"
    ).ap()
    # weights split by oc so each half is one contiguous 2KB/partition DMA
    w8_d = nc.dram_tensor("w8", [P, OC, NCH, 2, P], f8, kind="ExternalInput").ap()
    # thr replicated x128 on host: full 1KB/partition descriptors instead
    # of 8B ones (which waste ~3us of ring turns at packet round-robin)
    thr_d = nc.dram_tensor("thr", [P, OC, P], f32, kind="ExternalInput").ap()
    out_d = nc.dram_tensor(
        "out", [P, N_GROUPS, OC, GRP], f8, kind="ExternalOutput"
    ).ap()

    with tile.TileContext(nc) as tc, ExitStack() as ctx:
        const = ctx.enter_context(tc.tile_pool(name="const", bufs=1))
        io = ctx.enter_context(tc.tile_pool(name="io", bufs=1))
        outp = ctx.enter_context(tc.tile_pool(name="outp", bufs=1))
        psum = ctx.enter_context(tc.tile_pool(name="psum", bufs=8, space="PSUM"))

        w8_sb = const.tile([P, OC, NCH, 2, P], f8)
        thr_sb = const.tile([P, OC, P], f32)
        warm_x = const.tile([P, 2, GRP], f8)

        xt = {}
        for g in range(N_GROUPS):
            xt[g] = io.tile([P, NCH, 2, GRP], f8, name=f"x{g}", bufs=1)

        # warmup operand comes from one memset, not DMA, so the PE can
        # start burning its HAM ramp right after the preamble barrier
        nc.vector.memset(warm_x, 0.25)

        # ── DMA triggers, need-ordered across the two ring FIFOs ──
        # Both w8 halves go first, one per ring (the interleaved stream
        # needs oc0-c0 AND oc1-c0 immediately); g0 in quarters (2KB
        # descriptors, receipt per 2 chunks) so the stream can start while
        # g0 is still arriving; all other groups as hi/lo halves (4KB
        # descriptors) delivered in lockstep across the rings.
        H = NCH // 2
        engs = [nc.sync, nc.scalar]
        nc.sync.dma_start(out=w8_sb[:, 0], in_=w8_d[:, 0])
        nc.scalar.dma_start(out=w8_sb[:, 1], in_=w8_d[:, 1])
        for q in range(4):
            engs[q % 2].dma_start(
                out=xt[0][:, 2 * q : 2 * q + 2], in_=x8_d[:, 0, 2 * q : 2 * q + 2]
            )
        for g in range(1, N_GROUPS):
            nc.sync.dma_start(out=xt[g][:, :H], in_=x8_d[:, g, :H])
            nc.scalar.dma_start(out=xt[g][:, H:], in_=x8_d[:, g, H:])
            if g == 3:
                # thr needed only when g0's epilogue becomes psum-critical
                # (~25 us); late enough to stay off the critical supply path
                nc.scalar.dma_start(out=thr_sb, in_=thr_d)

        # ── PE warmup: data-independent DR MMs at cold pace (~0.43-0.52
        # us each) spanning ~4.3 us so HAM reaches K=8/8 before the first
        # real matmul. psum never read; slots recycle into the pool.
        wps = [psum.tile([P, GRP], f32, name="ps") for _ in range(2)]
        for i in range(N_WARM):
            nc.tensor.matmul(
                wps[i % 2], warm_x[:, :, :P], warm_x,
                start=True, stop=True, perf_mode=DR,
            )

        # ── main stream: 16 groups x 8 chunk-steps x 2 oc-passes ──
        # The oc passes are interleaved per chunk so fresh-x demand is a
        # steady ~296 GB/s (one 128KB chunk per 2 MMs) instead of 2x-supply
        # bursts during each oc0 pass — the rings then never fall behind.
        # The LAST group runs its oc passes sequentially so the oc0
        # epilogue + out drain ~1.7 us before the end (shorter tail), with
        # its out split per-oc across the two then-idle rings.
        for g in range(N_GROUPS):
            last = g == N_GROUPS - 1
            if not last:
                ob = outp.tile([P, OC, GRP], f8, name=f"ob{g}", bufs=1)
                pss = [psum.tile([P, GRP], f32, name="ps") for _ in range(OC)]
                for c in range(NCH):
                    for oc in range(OC):
                        nc.tensor.matmul(
                            pss[oc],
                            w8_sb[:, oc, c],
                            xt[g][:, c],
                            start=(c == 0),
                            stop=(c == NCH - 1),
                            perf_mode=DR,
                        )
                for oc in range(OC):
                    nc.vector.tensor_scalar(
                        ob[:, oc],
                        pss[oc],
                        thr_sb[:, oc, :1],
                        None,
                        mybir.AluOpType.is_ge,
                    )
                eng = nc.sync if g % 2 else nc.scalar
                eng.dma_start(out=out_d[:, g], in_=ob)
            else:
                ob_last = outp.tile([P, OC, GRP], f8, name="oblast", bufs=1)
                for oc in range(OC):
                    ps = psum.tile([P, GRP], f32, name="ps")
                    for c in range(NCH):
                        nc.tensor.matmul(
                            ps,
                            w8_sb[:, oc, c],
                            xt[g][:, c],
                            start=(c == 0),
                            stop=(c == NCH - 1),
                            perf_mode=DR,
                        )
                    if oc == 0:
                        nc.vector.tensor_scalar(
                            ob_last[:, 0],
                            ps,
                            thr_sb[:, 0, :1],
                            None,
                            mybir.AluOpType.is_ge,
                        )
                        nc.sync.dma_start(out=out_d[:, g, 0], in_=ob_last[:, 0])
                    else:
                        # final pass: halve the epilogue so the first out
                        # can trigger ~0.4us earlier and the two receipt
                        # round-trips overlap on the two idle rings
                        hg = GRP // 2
                        for h in range(2):
                            nc.vector.tensor_scalar(
                                ob_last[:, 1, h * hg : (h + 1) * hg],
                                ps[:, h * hg : (h + 1) * hg],
                                thr_sb[:, 1, :1],
                                None,
                                mybir.AluOpType.is_ge,
                            )
                            eng = nc.scalar if h == 0 else nc.sync
                            eng.dma_start(
                                out=out_d[:, g, 1, h * hg : (h + 1) * hg],
                                in_=ob_last[:, 1, h * hg : (h + 1) * hg],
                            )

    nc.compile()
    _CACHE["nc"] = nc
    return nc


def _prep_inputs(x, weight, bias, sign):
    """Host-side prep: fold sign into weights, build thresholds, split x into
    an e4m3 hi + e4m3 residual*64 pair in DoubleRow-interleaved layout."""
    f8np = ml_dtypes.float8_e4m3fn
    x = np.asarray(x, dtype=np.float32)
    weight = np.asarray(weight, dtype=np.float32)
    bias = np.asarray(bias, dtype=np.float32)
    sign = np.asarray(sign, dtype=np.float32).reshape(1, OUT_F)

    wp = sign.T * weight                      # [OUT_F, IN_F], ternary
    thr = (-sign[0] * bias - np.float32(0.5)).astype(np.float32)  # [OUT_F]
    thr2 = np.ascontiguousarray(
        np.repeat(thr.reshape(OC, P).T[:, :, None], P, axis=2)
    )  # [P, OC, P] replicated for full-size DMA descriptors

    # weights: [P, oc, chunk, j, 128]; chunks 0:4 = W' (ternary, exact in
    # e4m3), 4:8 = W'/64 (+-2^-6, exact in e4m3)
    wT = wp.T  # [IN_F, OUT_F]
    whi = wT.reshape(NCH // 2, 2, P, OUT_F).transpose(2, 0, 1, 3)
    wlo = (wT * np.float32(1.0 / 64.0)).reshape(NCH // 2, 2, P, OUT_F).transpose(
        2, 0, 1, 3
    )
    w8 = np.concatenate([whi, wlo], axis=1)   # [P, NCH, 2, OUT_F]
    w8 = np.ascontiguousarray(
        w8.reshape(P, NCH, 2, OC, P).transpose(0, 3, 1, 2, 4)
    ).astype(f8np)                            # [P, OC, NCH, 2, P]

    xhi8 = x.astype(f8np)
    xlo8 = ((x - xhi8.astype(np.float32)) * np.float32(64.0)).astype(f8np)

    in_maps = []
    for c in range(N_CORES):
        sl = slice(c * B_CORE, (c + 1) * B_CORE)
        hi = xhi8[sl].reshape(N_GROUPS, GRP, NCH // 2, 2, P).transpose(
            4, 0, 2, 3, 1
        )                                      # [P, g, 4, 2, GRP]
        lo = xlo8[sl].reshape(N_GROUPS, GRP, NCH // 2, 2, P).transpose(
            4, 0, 2, 3, 1
        )
        x8 = np.ascontiguousarray(np.concatenate([hi, lo], axis=2))
        in_maps.append({"x8": x8, "w8": w8, "thr": thr2})
    return in_maps


def _assemble(results):
    """[core][P, N_GROUPS, OC, GRP] fp8 -> [BATCH, OUT_F] fp32"""
    parts = []
    for r in results:
        a = (
            np.asarray(r["out"])
            .view(ml_dtypes.float8_e4m3fn)
            .astype(np.float32)
            .reshape(P, N_GROUPS, OC, GRP)
        )
        # out[b, o]: b = g*GRP + col, o = oc*P + p
        parts.append(
            a.transpose(1, 3, 2, 0).reshape(B_CORE, OUT_F)
        )
    return np.ascontiguousarray(np.concatenate(parts, axis=0))


def run(x, weight, bias, sign, trace=False):
    """Run the kernel; returns (output, BassKernelResults)."""
    from concourse.bass_utils import run_bass_kernel_spmd

    if not trace:
        os.environ["BASS_NEVER_TRACE"] = "1"
    else:
        os.environ.pop("BASS_NEVER_TRACE", None)

    nc = _build()
    in_maps = _prep_inputs(x, weight, bias, sign)
    res = run_bass_kernel_spmd(
        nc,
        in_maps,
        core_ids=list(range(N_CORES)),
        trace=trace,
    )
    return _assemble(res.results), res


def kernel(x, weight, bias, sign):
    out, _ = run(x, weight, bias, sign, trace=False)
    return out
